# revision 13
# baseline (speedup 1.0000x reference)
"""Trainium2 Bass kernel: single-head causal attention, data-parallel over batch.

Problem: x [4096, 64, 128] f32, Wq/Wk/Wv [128, 64] f32.
  q,k,v = x @ W*;  scores = q k^T / sqrt(128); causal softmax; out = attn @ v.

Sharding: batch 4096 -> 8 cores x 512 batches. Each core loops over 32
super-tiles of 16 batches (1024 rows of x).

Structure (v3):
  * A-trick: scores = x A x^T with A = Wq Wk^T / sqrt(C) folded on host.
  * 4KB-descriptor SWDGE cast-load: partition p holds 8 consecutive rows
    (f32->bf16 in the DMA). Row order inside the core is therefore a
    fixed permutation z = 16*i + i' <-> row = 8*(16m+i') + i within pair
    block m; the causal mask is host-permuted to match and the output
    DMA view un-permutes.
  * Merged SV matmul: per batch pair ONE matmul with stationary xT_pair
    and moving [Y'_pair | wv] (N=192) yields [scores^T-block | v-block].
    Y' = A^T x^T (2 big matmuls, stationary A, reused).
  * P4: ONE matmul per pair: stationary = masked E block (virtual
    diag(E_e, E_o) -- mask zeroes cross-batch terms), moving [v | ones]
    gives [O' | softmax-denominators].

Per-core pipeline (bf16 matmuls, fp32 PSUM):
  1. SWDGE DMA-cast x tile [128, 1024] f32->bf16 (128 x 4KB descriptors).
  2. 8 PE transposes -> x^T (PSUM bf16) -> SBUF (vector).
  3. Y' = A^T x^T: 2 matmuls N=512 into a single reused PSUM bank;
     scalar copy-cast into the YW staging buffer (wv slots prefilled).
  4. SV: 8 pair-matmuls N=192 -> psSV [scores | v].
  5. exp on ACT (PSUM->SBUF bf16), permuted-mask mult on vector.
  6. v copy to SBUF + ones col (vector/gpsimd).
  7. P4: 8 pair-matmuls [O' | sums].
  8. normalize: O = O' * recip(sums) (vector); DMA out (un-permuting view).
"""

import os
import numpy as np
import ml_dtypes
from contextlib import ExitStack

B, T, C, H = 4096, 64, 128, 64
N_CORES = 8
ST_B = 16                    # batches per super-tile
ROWS = ST_B * T              # 1024
B_CORE = B // N_CORES        # 512
N_ST = B_CORE // ST_B        # 32

_cached = {}


def _build_nc():
    import concourse.bass as bass
    import concourse.mybir as mybir
    import concourse.tile as tile
    from concourse import bacc

    F32 = mybir.dt.float32
    BF16 = mybir.dt.bfloat16

    nc = bacc.Bacc("TRN2", target_bir_lowering=False, debug=False)
    x_d = nc.dram_tensor("x", [B_CORE * T, C], F32, kind="ExternalInput").ap()
    a_d = nc.dram_tensor("a", [C, C], BF16, kind="ExternalInput").ap()
    wv_d = nc.dram_tensor("wv", [C, H], BF16, kind="ExternalInput").ap()
    id_d = nc.dram_tensor("ident", [C, C], BF16, kind="ExternalInput").ap()
    mk_d = nc.dram_tensor("mask", [128, 1024], BF16, kind="ExternalInput").ap()
    o_d = nc.dram_tensor("o", [B_CORE * T, H], F32, kind="ExternalOutput").ap()

    with tile.TileContext(nc) as tc, ExitStack() as ctx:
        sb = ctx.enter_context(tc.tile_pool(name="sb", bufs=2))
        ps = ctx.enter_context(tc.tile_pool(name="ps", bufs=1, space="PSUM"))
        psO = ctx.enter_context(tc.tile_pool(name="psO", bufs=1, space="PSUM"))
        cpool = ctx.enter_context(tc.tile_pool(name="const", bufs=1))

        a_sb = cpool.tile([C, C], BF16, tag="a")
        wv_sb = cpool.tile([C, H], BF16, tag="wv")
        id_sb = cpool.tile([C, C], BF16, tag="id")
        mk_sb = cpool.tile([128, 1024], BF16, tag="mk")
        nc.sync.dma_start(a_sb[:], a_d)
        nc.sync.dma_start(wv_sb[:], wv_d)
        nc.sync.dma_start(id_sb[:], id_d)
        nc.sync.dma_start(mk_sb[:], mk_d)

        # Two persistent YW staging buffers (manual double-buffer):
        # per pair m: cols [192m, 192m+128) = Y' pair block, [192m+128, 192m+192) = wv.
        yw_bufs = [cpool.tile([128, 8 * 192], BF16, tag=f"yw{i}", name=f"yw{i}")
                   for i in range(2)]
        for yw in yw_bufs:
            yw_v = yw[:].rearrange("c (m n) -> c m n", n=192)
            nc.vector.tensor_copy(
                yw_v[:, :, 128:192],
                wv_sb[:].unsqueeze(1).broadcast_to((C, 8, H)),
            )

        # x rows: partition p holds rows 8p..8p+8 of the super-tile (4KB descr)
        xv = x_d.rearrange("(S p r) c -> S p (r c)", p=128, r=8)
        # out: natural order (t-side of everything is kept natural)
        ov = o_d.rearrange("(S m par t) h -> S (par t) m h", m=8, par=2, t=64)

        for st in range(N_ST):
            yw = yw_bufs[st % 2]
            yw_v = yw[:].rearrange("c (m n) -> c m n", n=192)

            # ---- SWDGE cast-load x (f32 HBM -> bf16 SBUF), 4KB descriptors
            x_bf = sb.tile([128, ROWS], BF16, tag="x_bf")
            nc.gpsimd.dma_start(x_bf[:], xv[st])

            # ---- PSUM tile for SV results; its slack words [192:256) of each
            # 256-word group stage the 8 transpose outputs (disjoint ranges).
            sv_ps = ps.tile([128, 2048], F32, tag="sv")
            sv_v = sv_ps[:].rearrange("c (g h2 n) -> c g h2 n", h2=2, n=256)
            sv_bf = sv_ps[:].bitcast(BF16)  # [128, 4096] bf16 view

            # ---- 8 PE transposes -> slack chunk i at bf16 cols 512i+384
            for i in range(8):
                nc.tensor.transpose(
                    sv_bf[:, 512 * i + 384:512 * i + 512],
                    x_bf[:, 128 * i:128 * (i + 1)],
                    id_sb[:],
                )
            # permuting copy -> pair-major xT: col = 128m + 16i + i'
            # (z = 16i + i' within pair block m; holds row 8*(16m+i') + i)
            xT_sb = sb.tile([128, ROWS], BF16, tag="xT_sb")
            nc.vector.tensor_copy(
                xT_sb[:].rearrange("c (m i ip) -> c m i ip", m=8, i=8),
                sv_bf.rearrange("c (i r) -> c i r", i=8)[:, :, 384:512]
                .rearrange("c i (m ip) -> c m i ip", m=8),
            )

            # ---- Y' = A^T x^T, two N=512 matmuls (2 banks, back-to-back).
            # The moving operand streams xT columns in NATURAL row order via a
            # 4-D AP, so y cols land as n = 128m + 64bp + 8pp + i and the YW
            # scatter below is a single near-contiguous copy.
            xT_nat = xT_sb[:].rearrange(
                "c (m i bp pp) -> c m bp pp i", m=8, i=8, bp=2)
            y_ps = ps.tile([128, 1024], F32, tag="y")
            for h in range(2):
                nc.tensor.matmul(
                    y_ps[:, 512 * h:512 * h + 512],
                    a_sb[:],
                    xT_nat[:, 4 * h:4 * h + 4],
                    start=True, stop=True,
                )
            nc.scalar.copy(
                yw_v[:, :, 0:128],
                y_ps[:].rearrange("c (m n) -> c m n", n=128),
            )

            # ---- merged SV: per pair ONE matmul N=192 -> [scoresT | v]
            # psSV bank packing: pair m at word 512*(m//2) + 256*(m%2)
            for m in range(8):
                nc.tensor.matmul(
                    sv_v[:, m // 2, m % 2, 0:192],
                    xT_sb[:, 128 * m:128 * m + 128],
                    yw[:, 192 * m:192 * m + 192],
                    start=True, stop=True,
                )

            # ---- exp (ACT) then permuted causal mask (vector)
            E_raw = sb.tile([128, 1024], BF16, tag="Eraw")
            nc.scalar.activation(
                E_raw[:].rearrange("c (g h2 n) -> c g h2 n", h2=2, n=128),
                sv_v[:, :, :, 0:128],
                mybir.ActivationFunctionType.Exp,
            )
            E_sb = sb.tile([128, 1024], BF16, tag="E")
            nc.vector.tensor_tensor(
                out=E_sb[:], in0=E_raw[:], in1=mk_sb[:],
                op=mybir.AluOpType.mult,
            )

            # ---- v to SBUF + ones col
            v_sb = sb.tile([128, 8 * 66], BF16, tag="v_sb")
            v_sb_v = v_sb[:].rearrange("p (m z) -> p m z", z=66)
            nc.vector.tensor_copy(
                v_sb_v[:, :, 0:64].rearrange("p (g h2) z -> p g h2 z", g=4),
                sv_v[:, :, :, 128:192],
            )
            nc.gpsimd.memset(v_sb_v[:, :, 64:65], 1.0)

            # ---- P4: [O' | sums] per batch pair, ONE matmul each
            o_ps = psO.tile([128, 1024], F32, tag="o")
            for m in range(8):
                off = 512 * (m // 4) + 65 * (m % 4)
                nc.tensor.matmul(
                    o_ps[:, off:off + 65],
                    E_sb[:, 128 * m:128 * m + 128],
                    v_sb[:, 66 * m:66 * m + 65],
                    start=True, stop=True,
                )

            # ---- normalize: O = O' * recip(sums)
            opsv = o_ps[:].rearrange("p (B x) -> p B x", B=2)[:, :, 0:260]
            opsb = opsv.rearrange("p B (m z) -> p B m z", z=65)
            r_sb = sb.tile([128, 8], F32, tag="r")
            r_v = r_sb[:].rearrange("p (B m) -> p B m", B=2)
            nc.vector.reciprocal(r_v.unsqueeze(3), opsb[:, :, :, 64:65])
            o_sb = sb.tile([128, 512], F32, tag="o_sb")
            nc.vector.tensor_tensor(
                out=o_sb[:].rearrange("p (B m t) -> p B m t", B=2, t=64),
                in0=opsb[:, :, :, 0:64],
                in1=r_v.unsqueeze(3).broadcast_to((128, 2, 4, 64)),
                op=mybir.AluOpType.mult,
            )

            # ---- DMA out
            nc.sync.dma_start(ov[st], o_sb[:].rearrange("p (m h) -> p m h", h=64))

    nc.compile()
    return nc


def _host_inputs(x, Wq, Wk, Wv):
    bf = ml_dtypes.bfloat16
    a = np.ascontiguousarray((Wq @ Wk.T * (C ** -0.5)).astype(bf))
    wv_bf = np.ascontiguousarray(Wv.astype(bf))
    ident = np.eye(128, dtype=bf)
    # mixed-order causal mask: rows (s-side) are z-scrambled, cols natural.
    # z = 16i + 8bp + pp -> s = 8*pp + i, parity bp; col n -> t = n%64, n//64
    z = np.arange(128)
    bp_s = (z % 16) // 8
    s = 8 * (z % 8) + z // 16
    n = np.arange(128)
    bp_t = n // 64
    t = n % 64
    mask_pair = ((bp_s[:, None] == bp_t[None, :]) &
                 (s[:, None] <= t[None, :])).astype(np.float32)
    mask = np.ascontiguousarray(np.tile(mask_pair, (1, 8)).astype(bf))
    in_maps = []
    for c in range(N_CORES):
        shard = np.ascontiguousarray(
            x[c * B_CORE:(c + 1) * B_CORE].reshape(B_CORE * T, C)
        ).astype(np.float32)
        in_maps.append({
            "x": shard, "a": a, "wv": wv_bf,
            "ident": ident, "mask": mask,
        })
    return in_maps


def run(x, Wq, Wk, Wv, trace=False, **run_kwargs):
    from concourse import bass_utils

    if "nc" not in _cached:
        _cached["nc"] = _build_nc()
    nc = _cached["nc"]
    in_maps = _host_inputs(np.asarray(x), np.asarray(Wq),
                           np.asarray(Wk), np.asarray(Wv))
    res = bass_utils.run_bass_kernel_spmd(
        nc, in_maps, core_ids=list(range(N_CORES)), trace=trace, **run_kwargs
    )
    outs = [r["o"].reshape(B_CORE, T, H) for r in res.results]
    return np.concatenate(outs, axis=0), res


def kernel(x, Wq, Wk, Wv):
    out, _ = run(x, Wq, Wk, Wv, trace=False)
    return out


# revision 14
# speedup vs baseline: 1.7417x; 1.7417x over previous
"""Trainium2 Bass kernel: single-head causal attention, data-parallel over batch.

Problem: x [4096, 64, 128] f32, Wq/Wk/Wv [128, 64] f32.
  q,k,v = x @ W*;  scores = q k^T / sqrt(128); causal softmax; out = attn @ v.

Sharding: batch 4096 -> 8 cores x 512 batches. Each core loops over 32
super-tiles of 16 batches (1024 rows of x).

Structure (v5):
  * A-trick: scores = x A x^T with A = Wq Wk^T / sqrt(C) folded on host.
  * 4KB-descriptor SWDGE cast-load: partition p holds 8 consecutive rows
    (f32->bf16 in the DMA). The in-core row order is the fixed permutation
    z = 16i + i' <-> row 8*(16m+i') + i within pair block m; the s-side
    stays z-scrambled (host mask absorbs it), the t-side is natural.
  * Y' = A^T x^T: moving operand streams columns in NATURAL row order via
    a 4-D access pattern, so Y' columns are natural and no scatter-copy
    is needed anywhere.
  * P_S: per pair ONE matmul (stationary xT pair block, moving Y' pair
    block, N=128) -> [z_s, t] score block (off-parity garbage masked).
  * P4: per pair ONE matmul: stationary masked E block (virtual
    diag(E_e, E_o)), moving [v | ones] -> [O' | softmax denominators].

Per-core pipeline (bf16 matmuls, fp32 PSUM):
  1. SWDGE DMA-cast x tile [128, 1024] f32->bf16 (128 x 4KB descriptors).
  2. 8 PE transposes -> x^T (PSUM bf16) -> pair-major SBUF (vector).
  3. Y' matmuls (N=512 x2) -> PSUM -> SBUF bf16 (scalar, contiguous).
  4. v = x wv: 8 matmuls (stationary xT pair blocks, rhs wv).
  5. P_S: 8 pair-matmuls -> sc_ps [128, 1024].
  6. exp on ACT (PSUM->SBUF bf16), mixed-order causal mask on vector.
  7. P4: 8 pair-matmuls [O' | sums].
  8. normalize: O = O' * recip(sums) (vector); DMA out.
"""

import os
import numpy as np
import ml_dtypes
from contextlib import ExitStack

B, T, C, H = 4096, 64, 128, 64
N_CORES = 8
ST_B = 16                    # batches per super-tile
ROWS = ST_B * T              # 1024
B_CORE = B // N_CORES        # 512
N_ST = B_CORE // ST_B        # 32

_cached = {}


def _build_nc():
    import concourse.bass as bass
    import concourse.mybir as mybir
    import concourse.tile as tile
    from concourse import bacc

    F32 = mybir.dt.float32
    BF16 = mybir.dt.bfloat16

    nc = bacc.Bacc("TRN2", target_bir_lowering=False, debug=False)
    x_d = nc.dram_tensor("x", [B_CORE * T, C], F32, kind="ExternalInput").ap()
    a_d = nc.dram_tensor("a", [C, C], BF16, kind="ExternalInput").ap()
    wv_d = nc.dram_tensor("wv", [C, H], BF16, kind="ExternalInput").ap()
    id_d = nc.dram_tensor("ident", [C, C], BF16, kind="ExternalInput").ap()
    mk_d = nc.dram_tensor("mask", [128, 1024], BF16, kind="ExternalInput").ap()
    o_d = nc.dram_tensor("o", [B_CORE * T, H], F32, kind="ExternalOutput").ap()

    with tile.TileContext(nc) as tc, ExitStack() as ctx:
        sb = ctx.enter_context(tc.tile_pool(name="sb", bufs=2))
        ps = ctx.enter_context(tc.tile_pool(name="ps", bufs=1, space="PSUM"))
        psO = ctx.enter_context(tc.tile_pool(name="psO", bufs=1, space="PSUM"))
        cpool = ctx.enter_context(tc.tile_pool(name="const", bufs=1))

        a_sb = cpool.tile([C, C], BF16, tag="a")
        wv_sb = cpool.tile([C, H], BF16, tag="wv")
        id_sb = cpool.tile([C, C], BF16, tag="id")
        mk_sb = cpool.tile([128, 1024], BF16, tag="mk")
        nc.sync.dma_start(a_sb[:], a_d)
        nc.sync.dma_start(wv_sb[:], wv_d)
        nc.sync.dma_start(id_sb[:], id_d)
        nc.sync.dma_start(mk_sb[:], mk_d)

        # x rows: partition p holds rows 8p..8p+8 of the super-tile (4KB descr)
        xv = x_d.rearrange("(S p r) c -> S p (r c)", p=128, r=8)
        # out: natural order (t-side of everything is natural)
        ov = o_d.rearrange("(S m par t) h -> S (par t) m h", m=8, par=2, t=64)

        for st in range(N_ST):
            # ---- SWDGE cast-load x (f32 HBM -> bf16 SBUF), 4KB descriptors
            x_bf = sb.tile([128, ROWS], BF16, tag="x_bf")
            nc.gpsimd.dma_start(x_bf[:], xv[st])

            # ---- 8 PE transposes -> xT in PSUM (bf16)
            xT_ps = ps.tile([128, ROWS // 2], F32, tag="xT")
            xT_ps_bf = xT_ps[:].bitcast(BF16)
            for i in range(8):
                nc.tensor.transpose(
                    xT_ps_bf[:, 128 * i:128 * (i + 1)],
                    x_bf[:, 128 * i:128 * (i + 1)],
                    id_sb[:],
                )
            # permuting copy -> pair-major xT: col = 128m + 16i + i'
            # (z = 16i + i' within pair block m; holds row 8*(16m+i') + i)
            xT_sb = sb.tile([128, ROWS], BF16, tag="xT_sb")
            nc.vector.tensor_copy(
                xT_sb[:].rearrange("c (m i ip) -> c m i ip", m=8, i=8),
                xT_ps_bf.rearrange("c (i m ip) -> c m i ip", i=8, m=8),
            )

            # ---- Y' = A^T x^T, two N=512 matmuls (natural-order streaming)
            # y col = 128m + 64bp + 8pp + i  (natural: t = 8pp + i, parity bp)
            xT_nat = xT_sb[:].rearrange(
                "c (m i bp pp) -> c m bp pp i", m=8, i=8, bp=2)
            y_ps = ps.tile([128, 1024], F32, tag="y")
            for h in range(2):
                nc.tensor.matmul(
                    y_ps[:, 512 * h:512 * h + 512],
                    a_sb[:],
                    xT_nat[:, 4 * h:4 * h + 4],
                    start=True, stop=True,
                )
            y_sb = sb.tile([128, 1024], BF16, tag="y_sb")
            nc.scalar.copy(y_sb[:], y_ps[:])

            # ---- v = x @ wv (stationary xT pair blocks, rhs wv) -> v [z, h]
            v_ps = ps.tile([128, 512], F32, tag="v")
            for m in range(8):
                nc.tensor.matmul(
                    v_ps[:, 64 * m:64 * m + 64],
                    xT_sb[:, 128 * m:128 * m + 128],
                    wv_sb[:],
                    start=True, stop=True,
                )
            v_sb = sb.tile([128, 8 * 66], BF16, tag="v_sb")
            v_sb_v = v_sb[:].rearrange("p (m z) -> p m z", z=66)
            nc.vector.tensor_copy(
                v_sb_v[:, :, 0:64],
                v_ps[:].rearrange("p (m t) -> p m t", t=64),
            )
            nc.gpsimd.memset(v_sb_v[:, :, 64:65], 1.0)

            # ---- P_S: per pair ONE matmul -> sc[z_s, t] block
            sc_ps = ps.tile([128, 1024], F32, tag="sc")
            for m in range(8):
                nc.tensor.matmul(
                    sc_ps[:, 128 * m:128 * m + 128],
                    xT_sb[:, 128 * m:128 * m + 128],
                    y_sb[:, 128 * m:128 * m + 128],
                    start=True, stop=True,
                )

            # ---- exp (ACT) then mixed-order causal mask (vector)
            E_raw = sb.tile([128, 1024], BF16, tag="Eraw")
            nc.scalar.activation(
                E_raw[:], sc_ps[:], mybir.ActivationFunctionType.Exp
            )
            E_sb = sb.tile([128, 1024], BF16, tag="E")
            nc.vector.tensor_tensor(
                out=E_sb[:], in0=E_raw[:], in1=mk_sb[:],
                op=mybir.AluOpType.mult,
            )

            # ---- P4: [O' | sums] per batch pair, ONE matmul each
            o_ps = psO.tile([128, 1024], F32, tag="o")
            for m in range(8):
                off = 512 * (m // 4) + 65 * (m % 4)
                nc.tensor.matmul(
                    o_ps[:, off:off + 65],
                    E_sb[:, 128 * m:128 * m + 128],
                    v_sb[:, 66 * m:66 * m + 65],
                    start=True, stop=True,
                )

            # ---- normalize: O = O' * recip(sums)
            opsv = o_ps[:].rearrange("p (B x) -> p B x", B=2)[:, :, 0:260]
            opsb = opsv.rearrange("p B (m z) -> p B m z", z=65)
            r_sb = sb.tile([128, 8], F32, tag="r")
            r_v = r_sb[:].rearrange("p (B m) -> p B m", B=2)
            nc.vector.reciprocal(r_v.unsqueeze(3), opsb[:, :, :, 64:65])
            o_sb = sb.tile([128, 512], F32, tag="o_sb")
            nc.vector.tensor_tensor(
                out=o_sb[:].rearrange("p (B m t) -> p B m t", B=2, t=64),
                in0=opsb[:, :, :, 0:64],
                in1=r_v.unsqueeze(3).broadcast_to((128, 2, 4, 64)),
                op=mybir.AluOpType.mult,
            )

            # ---- DMA out
            nc.sync.dma_start(ov[st], o_sb[:].rearrange("p (m h) -> p m h", h=64))

    nc.compile()
    return nc


def _host_inputs(x, Wq, Wk, Wv):
    bf = ml_dtypes.bfloat16
    a = np.ascontiguousarray((Wq @ Wk.T * (C ** -0.5)).astype(bf))
    wv_bf = np.ascontiguousarray(Wv.astype(bf))
    ident = np.eye(128, dtype=bf)
    # mixed-order causal mask: rows (s-side) are z-scrambled, cols natural.
    # z = 16i + 8bp + pp -> s = 8*pp + i, parity bp; col n -> t = n%64, n//64
    z = np.arange(128)
    bp_s = (z % 16) // 8
    s = 8 * (z % 8) + z // 16
    n = np.arange(128)
    bp_t = n // 64
    t = n % 64
    mask_pair = ((bp_s[:, None] == bp_t[None, :]) &
                 (s[:, None] <= t[None, :])).astype(np.float32)
    mask = np.ascontiguousarray(np.tile(mask_pair, (1, 8)).astype(bf))
    in_maps = []
    for c in range(N_CORES):
        shard = np.ascontiguousarray(
            x[c * B_CORE:(c + 1) * B_CORE].reshape(B_CORE * T, C)
        ).astype(np.float32)
        in_maps.append({
            "x": shard, "a": a, "wv": wv_bf,
            "ident": ident, "mask": mask,
        })
    return in_maps


def run(x, Wq, Wk, Wv, trace=False, **run_kwargs):
    from concourse import bass_utils

    if "nc" not in _cached:
        _cached["nc"] = _build_nc()
    nc = _cached["nc"]
    in_maps = _host_inputs(np.asarray(x), np.asarray(Wq),
                           np.asarray(Wk), np.asarray(Wv))
    res = bass_utils.run_bass_kernel_spmd(
        nc, in_maps, core_ids=list(range(N_CORES)), trace=trace, **run_kwargs
    )
    outs = [r["o"].reshape(B_CORE, T, H) for r in res.results]
    return np.concatenate(outs, axis=0), res


def kernel(x, Wq, Wk, Wv):
    out, _ = run(x, Wq, Wk, Wv, trace=False)
    return out


# revision 17
# speedup vs baseline: 2.6021x; 1.4940x over previous
"""Trainium2 Bass kernel: single-head causal attention, data-parallel over batch.

Problem: x [4096, 64, 128] f32, Wq/Wk/Wv [128, 64] f32.
  q,k,v = x @ W*;  scores = q k^T / sqrt(128); causal softmax; out = attn @ v.

Sharding: batch 4096 -> 8 cores x 512 batches. Each core loops over 32
super-tiles of 16 batches (1024 rows of x).

Key restructuring vs a naive q/k/v pipeline:
  * A-trick: scores = x A x^T with A = Wq Wk^T / sqrt(C) folded on host.
    One [128x128] stationary (A^T) + 2 big matmuls per tile replace the
    whole q/k projection stage.
  * SWDGE cast-DMA loads x f32->bf16 straight from HBM (no engine cast).
  * Batch-pair packing: each pair of batches shares one 128-row block.
    P_S computes a [128,128] block = diag(scores_e^T, scores_o^T) plus
    off-diagonal garbage in ONE matmul; the causal mask (kron(I2, tri))
    zeroes the garbage after exp. P4 then uses diag(E_e, E_o) as a single
    128-col stationary to produce both batches' [O'|sums] in ONE matmul.

Per-core pipeline (bf16 matmuls, fp32 PSUM):
  1. SWDGE DMA-cast x tile [128, 1024] f32->bf16.
  2. 8 PE transposes -> x^T (PSUM, bf16) -> SBUF (vector).
  3. Y = A x^T: 2 matmuls N=512 (stationary A^T) -> PSUM -> SBUF bf16 (scalar).
  4. v = x W~v: 8 matmuls (stationary x^T chunks, rhs wv) -> SBUF bf16 + ones col.
  5. P_S: 8 pair-matmuls -> sc_ps [128, 1024] (diag-packed scores^T).
  6. exp on ACT (PSUM->SBUF bf16), mask-mult on vector (zeroes garbage).
  7. P4: 8 pair-matmuls [O'|sums] = E^T.T @ [V|ones].
  8. normalize: O = O' * recip(sums); DMA out.
"""

import os
import numpy as np
import ml_dtypes
from contextlib import ExitStack

B, T, C, H = 4096, 64, 128, 64
N_CORES = 8
ST_B = 16                    # batches per super-tile
ROWS = ST_B * T              # 1024
B_CORE = B // N_CORES        # 512
N_ST = B_CORE // ST_B        # 32

_cached = {}


def _build_nc():
    import concourse.bass as bass
    import concourse.mybir as mybir
    import concourse.tile as tile
    from concourse import bacc

    F32 = mybir.dt.float32
    BF16 = mybir.dt.bfloat16

    nc = bacc.Bacc("TRN2", target_bir_lowering=False, debug=False)
    x_d = nc.dram_tensor("x", [B_CORE * T, C], F32, kind="ExternalInput").ap()
    at_d = nc.dram_tensor("at", [C, C], BF16, kind="ExternalInput").ap()
    wv_d = nc.dram_tensor("wv", [C, H], BF16, kind="ExternalInput").ap()
    id_d = nc.dram_tensor("ident", [C, C], BF16, kind="ExternalInput").ap()
    mk_d = nc.dram_tensor("mask", [128, 1024], BF16, kind="ExternalInput").ap()
    o_d = nc.dram_tensor("o", [B_CORE * T, H], F32, kind="ExternalOutput").ap()

    with tile.TileContext(nc) as tc, ExitStack() as ctx:
        sb = ctx.enter_context(tc.tile_pool(name="sb", bufs=4))
        ps = ctx.enter_context(tc.tile_pool(name="ps", bufs=1, space="PSUM"))
        psO = ctx.enter_context(tc.tile_pool(name="psO", bufs=1, space="PSUM"))
        cpool = ctx.enter_context(tc.tile_pool(name="const", bufs=1))

        at_sb = cpool.tile([C, C], BF16, tag="at")
        wv_sb = cpool.tile([C, H], BF16, tag="wv")
        id_sb = cpool.tile([C, C], BF16, tag="id")
        mk_sb = cpool.tile([128, 1024], BF16, tag="mk")
        nc.sync.dma_start(at_sb[:], at_d)
        nc.sync.dma_start(wv_sb[:], wv_d)
        nc.sync.dma_start(id_sb[:], id_d)
        nc.sync.dma_start(mk_sb[:], mk_d)

        xv = x_d.rearrange("(S n p) c -> S p n c", n=8, p=128)
        ov = o_d.rearrange("(S m par t) h -> S (par t) m h", m=8, par=2, t=64)

        for st in range(N_ST):
            # ---- SWDGE cast-load x (f32 HBM -> bf16 SBUF)
            x_bf = sb.tile([128, ROWS], BF16, tag="x_bf")
            nc.gpsimd.dma_start(
                x_bf[:].rearrange("p (n c) -> p n c", n=8), xv[st]
            )

            # ---- 8 PE transposes -> xT in PSUM (bf16), then copy to SBUF
            xT_ps = ps.tile([128, ROWS // 2], F32, tag="xT")
            xT_ps_bf = xT_ps[:].bitcast(BF16)
            for i in range(8):
                nc.tensor.transpose(
                    xT_ps_bf[:, 128 * i:128 * (i + 1)],
                    x_bf[:, 128 * i:128 * (i + 1)],
                    id_sb[:],
                )
            xT_sb = sb.tile([128, ROWS], BF16, tag="xT_sb")
            nc.vector.tensor_copy(xT_sb[:], xT_ps_bf)

            # ---- Y = A x^T (stationary A^T), 2 matmuls N=512
            y_ps = ps.tile([128, 1024], F32, tag="y")
            for half in range(2):
                nc.tensor.matmul(
                    y_ps[:, 512 * half:512 * half + 512],
                    at_sb[:],
                    xT_sb[:, 512 * half:512 * half + 512],
                    start=True, stop=True,
                )
            y_sb = sb.tile([128, 1024], BF16, tag="y_sb")
            nc.scalar.copy(y_sb[:], y_ps[:])

            # ---- v = x @ wv (stationary x^T chunks, rhs wv)
            v_ps = ps.tile([128, 512], F32, tag="v")
            for m in range(8):
                nc.tensor.matmul(
                    v_ps[:, 64 * m:64 * m + 64],
                    xT_sb[:, 128 * m:128 * m + 128],
                    wv_sb[:],
                    start=True, stop=True,
                )
            v_sb = sb.tile([128, 8 * 66], BF16, tag="v_sb")
            v_sb_v = v_sb[:].rearrange("p (m z) -> p m z", z=66)
            nc.vector.tensor_copy(
                v_sb_v[:, :, 0:64],
                v_ps[:].rearrange("p (m t) -> p m t", t=64),
            )
            nc.gpsimd.memset(v_sb_v[:, :, 64:65], 1.0)

            # ---- P_S: diag-packed scores^T per batch pair, ONE matmul each
            sc_ps = ps.tile([128, 1024], F32, tag="sc")
            for m in range(8):
                nc.tensor.matmul(
                    sc_ps[:, 128 * m:128 * m + 128],
                    y_sb[:, 128 * m:128 * m + 128],
                    xT_sb[:, 128 * m:128 * m + 128],
                    start=True, stop=True,
                )

            # ---- exp (ACT) then multiplicative causal mask (vector)
            E_raw = sb.tile([128, 1024], BF16, tag="Eraw")
            nc.scalar.activation(
                E_raw[:], sc_ps[:], mybir.ActivationFunctionType.Exp
            )
            E_sb = sb.tile([128, 1024], BF16, tag="E")
            nc.vector.tensor_tensor(
                out=E_sb[:], in0=E_raw[:], in1=mk_sb[:],
                op=mybir.AluOpType.mult,
            )

            # ---- P4: [O' | sums] per batch pair, ONE matmul each
            o_ps = psO.tile([128, 1024], F32, tag="o")
            for m in range(8):
                off = 512 * (m // 4) + 65 * (m % 4)
                nc.tensor.matmul(
                    o_ps[:, off:off + 65],
                    E_sb[:, 128 * m:128 * m + 128],
                    v_sb[:, 66 * m:66 * m + 65],
                    start=True, stop=True,
                )

            # ---- normalize: O = O' * recip(sums)
            opsv = o_ps[:].rearrange("p (B x) -> p B x", B=2)[:, :, 0:260]
            opsb = opsv.rearrange("p B (m z) -> p B m z", z=65)
            r_sb = sb.tile([128, 8], F32, tag="r")
            r_v = r_sb[:].rearrange("p (B m) -> p B m", B=2)
            nc.vector.reciprocal(r_v.unsqueeze(3), opsb[:, :, :, 64:65])
            o_sb = sb.tile([128, 512], F32, tag="o_sb")
            nc.vector.tensor_tensor(
                out=o_sb[:].rearrange("p (B m t) -> p B m t", B=2, t=64),
                in0=opsb[:, :, :, 0:64],
                in1=r_v.unsqueeze(3).broadcast_to((128, 2, 4, 64)),
                op=mybir.AluOpType.mult,
            )

            # ---- DMA out
            nc.sync.dma_start(ov[st], o_sb[:].rearrange("p (m h) -> p m h", h=64))

    nc.compile()
    return nc


def _host_inputs(x, Wq, Wk, Wv):
    bf = ml_dtypes.bfloat16
    at = np.ascontiguousarray((Wk @ Wq.T * (C ** -0.5)).astype(bf))
    wv_bf = np.ascontiguousarray(Wv.astype(bf))
    ident = np.eye(128, dtype=bf)
    tri = np.triu(np.ones((T, T), dtype=np.float32))  # [s, t]: 1 if s <= t
    mask_pair = np.kron(np.eye(2, dtype=np.float32), tri)  # [128, 128]
    mask = np.ascontiguousarray(np.tile(mask_pair, (1, 8)).astype(bf))
    in_maps = []
    for c in range(N_CORES):
        shard = np.ascontiguousarray(
            x[c * B_CORE:(c + 1) * B_CORE].reshape(B_CORE * T, C)
        ).astype(np.float32)
        in_maps.append({
            "x": shard, "at": at, "wv": wv_bf,
            "ident": ident, "mask": mask,
        })
    return in_maps


def run(x, Wq, Wk, Wv, trace=False, **run_kwargs):
    from concourse import bass_utils

    if "nc" not in _cached:
        _cached["nc"] = _build_nc()
    nc = _cached["nc"]
    in_maps = _host_inputs(np.asarray(x), np.asarray(Wq),
                           np.asarray(Wk), np.asarray(Wv))
    res = bass_utils.run_bass_kernel_spmd(
        nc, in_maps, core_ids=list(range(N_CORES)), trace=trace, **run_kwargs
    )
    outs = [r["o"].reshape(B_CORE, T, H) for r in res.results]
    return np.concatenate(outs, axis=0), res


def kernel(x, Wq, Wk, Wv):
    out, _ = run(x, Wq, Wk, Wv, trace=False)
    return out
